# revision 22
# baseline (speedup 1.0000x reference)
"""Trainium2 Bass kernel for nn_Attention_Rel_Scl (B=4,S=1024,E=1024,H=16).

Sharding: 8 cores = (batch b, head-half hg). Core c = 2*b + hg computes, for
batch b, heads 8*hg..8*hg+7 over the FULL sequence:
  out[:, 512*hg:512*hg+512] = LN-half of
      concat_h[ (softmax(q k^T / 32) + relbias_h) @ v_h ]
LayerNorm needs full-E row stats, so core pairs (2b, 2b+1) exchange per-row
partial bn_stats via a tiny AllGather (12 KB) and each normalizes its own 512
columns; chunk 0's collective hides under chunk 1's compute.

fp8 acceleration (vs the f32r/bf16 baseline): Q/K projections, scores and
softmax@V run as fp8e4m3 DoubleRow matmuls (2 contraction rows per PE row =
0.5 cycles/row). Numeric scaling: Wq/Wk are scaled x16 on host so their fp8
encoding stays in normal range; the resulting x256 on logits is folded into
the exp scale (1/(32*256)). bias@V stays bf16 (table entries ~0.02 make any
fp8 path exceed the 2e-2 error budget; measured). V is projected in f32r,
stored bf16 (bias@V) and re-quantized to fp8 (softmax@V's DR operand).
Scores contract over d=64 only, so the DR pair dim uses a zero plane
(ktq8 plane 8) as the second k-subtile; heads 2m/2m+1 sit on partitions
0-63/64-127 of the pair tiles.

Engine budget (cost model): PE ~75us (bias@V bf16 is the largest term),
ACT exp (64x [128,1024]) ~69us, DVE copies/combines/LN ~60us. The schedule
software-pipelines PV one-two pairs behind scores so the exp stream never
starves; the last collective is emitted before LN(0) so LN(0) hides under
it. A short PE warmup spin covers the input-DMA window so the first real
matmuls run at full p-state clock.
"""

import dataclasses
import sys

if "/opt/trn_rl_repo" not in sys.path:
    sys.path.insert(0, "/opt/trn_rl_repo")

import numpy as np
import ml_dtypes

import concourse.bass as bass
import concourse.mybir as mybir
import concourse.tile as tile
from concourse import bacc
from concourse.bass_utils import run_bass_kernel_spmd
from concourse.masks import make_identity

B, S, E, H = 4, 1024, 1024, 16
D = E // H          # 64
HC = H // 2         # 8 heads per core
EC = HC * D         # 512 output columns per core
NK = E // 128       # 8 contraction blocks
NQB = S // 128      # 8 query blocks
SCALE = float(E) ** -0.5
WS = 16.0           # host scale on Wq/Wk before fp8
EXPSCALE = SCALE / (WS * WS)
LN_EPS = 1e-5
VBW = 1920          # Toeplitz window width

F32 = mybir.dt.float32
F32R = mybir.dt.float32r
BF16 = mybir.dt.bfloat16
FP8 = mybir.dt.float8e4
DR = mybir.MatmulPerfMode.DoubleRow

_cache = {}


def _pair_ap(t, p0, plane, gap, off, width):
    """[64, 2, width] DR operand: partitions p0..p0+64, planes (plane,
    plane+gap) of a [128, 9, S] tile, free offset off."""
    base = t[p0:p0 + 64, plane, off:off + width]
    return dataclasses.replace(base, ap=[base.ap[0], [gap * S, 2], [1, width]])


def _build_nc():
    nc = bacc.Bacc("TRN2", target_bir_lowering=False, debug=False, num_devices=8)

    xT = nc.dram_tensor("xT", [E, S], F32R, kind="ExternalInput").ap()
    x8 = nc.dram_tensor("x8", [E, S], FP8, kind="ExternalInput").ap()
    wkq8 = nc.dram_tensor("wkq8", [8, 128, NK, 128], FP8, kind="ExternalInput").ap()
    wv = nc.dram_tensor("wv", [128, NK, EC], F32R, kind="ExternalInput").ap()
    tbl16 = nc.dram_tensor("tbl16", [HC, 128, VBW], BF16, kind="ExternalInput").ap()
    y = nc.dram_tensor("y", [S, EC], F32, kind="ExternalOutput").ap()
    cc = [
        (
            nc.dram_tensor(f"cc_in{i}", [128, 24], F32).ap(),
            nc.dram_tensor(f"cc_out{i}", [256, 24], F32).ap(),
        )
        for i in range(2)
    ]

    with tile.TileContext(nc) as tc:
        _emit(nc, tc, xT, x8, wkq8, wv, tbl16, y, cc)
    nc.finalize()
    return nc


def _emit(nc, tc, xT, x8, wkq8, wv, tbl16, y, cc):
    import contextlib

    ctx = contextlib.ExitStack()
    with ctx:
        singles = ctx.enter_context(tc.tile_pool(name="singles", bufs=1))
        epool = ctx.enter_context(tc.tile_pool(name="epool", bufs=6))
        upool = ctx.enter_context(tc.tile_pool(name="upool", bufs=4))
        small = ctx.enter_context(tc.tile_pool(name="small", bufs=6))
        pp = ctx.enter_context(tc.tile_pool(name="pp", bufs=2, space="PSUM"))
        pst = ctx.enter_context(tc.tile_pool(name="pst", bufs=2, space="PSUM"))
        pu = ctx.enter_context(tc.tile_pool(name="pu", bufs=2, space="PSUM"))

        # ---- resident tiles ---------------------------------------------
        # kt planes 0-3 (pair m), qt planes 4-7, plane 8 = zeros (DR filler)
        ktq8 = singles.tile([128, 9, S], FP8)
        nc.gpsimd.memset(ktq8[:, 8, :], 0.0)

        VP = D + 2                                     # padded: HC*VP % 16 == 0
        v8 = singles.tile([128, NK, HC, VP], FP8)      # slot j = s-block 7-j
        nc.vector.memset(v8[:, :, :, D:D + 1], 1.0)
        v16 = singles.tile([128, NK, HC, D], BF16)     # bf16 V for bias@V

        ident = singles.tile([128, 128], BF16)
        make_identity(nc, ident)
        eps_t = singles.tile([128, 1], F32)
        nc.vector.memset(eps_t, LN_EPS)

        # PE p-state warmup: keep the PE continuously busy through the
        # input-DMA window so the first real matmuls run at full clock
        # (the cost model ramps over ~3us of continuous execution).
        warm = pu.tile([128, 128], BF16, tag="u", name="warm")
        for _ in range(30):
            nc.tensor.transpose(warm, ident, ident)

        out_sb = singles.tile([128, NQB, EC], F32)      # 2 MB

        # DMA order = transfer order: first-needed first. Pair-0 weights,
        # then all of x8 (first kq gate), remaining weights, xT+wv (V path),
        # then per-head bias windows.
        w8all = singles.tile([128, 8, NK, 128], FP8)    # 1 MB
        x8_sb = singles.tile([128, NK, S], FP8)         # 1 MB
        xT_sb = singles.tile([128, NK, S], F32R)        # 4 MB
        wv_sb = singles.tile([128, NK, EC], F32R)       # 2 MB
        vbs_all = singles.tile([128, HC, VBW], BF16)    # 3.8 MB
        vbs = [vbs_all[:, h, :] for h in range(HC)]
        for i in (0, 4):
            nc.scalar.dma_start(out=w8all[:, i], in_=wkq8[i])
        nc.sync.dma_start(
            out=x8_sb,
            in_=bass.AP(tensor=x8.tensor, offset=x8.offset,
                        ap=[[S, 128], [128 * S, NK], [1, S]]),
        )
        for i in (1, 5, 2, 6, 3, 7):
            nc.scalar.dma_start(out=w8all[:, i], in_=wkq8[i])
        nc.scalar.dma_start(out=wv_sb, in_=wv)

        def load_xt_cols(mb):
            # s-column block: xT_sb[:, :, 128mb:128mb+128] across all k planes
            nc.sync.dma_start(
                out=xT_sb[:, :, 128 * mb:128 * (mb + 1)],
                in_=bass.AP(tensor=xT.tensor, offset=xT.offset + 128 * mb,
                            ap=[[S, 128], [128 * S, NK], [1, 128]]),
            )

        for mb in range(4):
            load_xt_cols(mb)
        for h in (0, 1):
            nc.sync.dma_start(out=vbs_all[:, h, :], in_=tbl16[h])
        for mb in range(4, 8):
            load_xt_cols(mb)
        for h in range(2, HC):
            nc.sync.dma_start(out=vbs_all[:, h, :], in_=tbl16[h])

        # ---- phases ------------------------------------------------------
        def emit_kq(m):
            for dst in range(2):            # 0 = kt, 1 = qt
                w = w8all[:, dst * 4 + m]
                plane = 4 * dst + m
                for n in range(2):
                    ps = pp.tile([128, 512], F32, tag="pp", name=f"pk{m}{dst}{n}")
                    for kp in range(4):
                        nc.tensor.matmul(
                            ps, lhsT=w[:, 2 * kp:2 * kp + 2, :],
                            rhs=x8_sb[:, 2 * kp:2 * kp + 2, 512 * n:512 * (n + 1)],
                            start=(kp == 0), stop=(kp == 3), perf_mode=DR,
                        )
                    nc.vector.tensor_copy(
                        out=ktq8[:, plane, 512 * n:512 * (n + 1)], in_=ps)

        def emit_v(blocks):
            for mb in blocks:     # s block
                ps = pp.tile([128, 512], F32, tag="pp", name=f"pv{mb}")
                for k in range(NK):
                    nc.tensor.matmul(
                        ps, lhsT=xT_sb[:, k, 128 * mb:128 * (mb + 1)],
                        rhs=wv_sb[:, k, :],
                        start=(k == 0), stop=(k == NK - 1),
                    )
                nc.vector.tensor_copy(
                    out=v16[:, 7 - mb, :, :],
                    in_=ps.rearrange("p (h d) -> p h d", d=D),
                )
                nc.gpsimd.tensor_copy(
                    out=v8[:, 7 - mb, :, 0:D], in_=v16[:, 7 - mb, :, :])

        def emit_scores_exp(m, qch):
            # head 2m on partitions 0-63, head 2m+1 on 64-127; DR pair dim =
            # (data plane, zero plane 8). st halves ordered so one exp writes
            # e8 slots (6-2kp, 7-2kp) = k-blocks (2kp+1, 2kp).
            q0 = 512 * qch
            e_pair = [
                epool.tile([128, NK, 512], FP8, tag="eh", name=f"e{m}{qch}{hl}")
                for hl in range(2)
            ]
            for hl in range(2):
                p0 = 64 * hl
                for kp in range(4):
                    st = pst.tile([128, 1024], F32, tag="st",
                                  name=f"st{m}{qch}{kp}{hl}")
                    for kh in range(2):
                        kb = 2 * kp + kh
                        nc.tensor.matmul(
                            st[:, 512 * (1 - kh):512 * (2 - kh)],
                            lhsT=_pair_ap(ktq8, p0, m, 8 - m, 128 * kb, 128),
                            rhs=_pair_ap(ktq8, p0, 4 + m, 4 - m, q0, 512),
                            start=True, stop=True, perf_mode=DR,
                        )
                    nc.scalar.activation(
                        out=e_pair[hl][:, 6 - 2 * kp:8 - 2 * kp, :].rearrange(
                            "p a b -> p (a b)"),
                        in_=st,
                        func=mybir.ActivationFunctionType.Exp,
                        scale=EXPSCALE,
                    )
            return e_pair

        def emit_pv(m, qch, e_pair, last=False, stats_sb=None):
            q0 = 512 * qch
            cp = nc.scalar.copy if last else None
            # bias@V (bf16): needs only v16 + vbar, keeps PE busy during exps.
            # slot j holds s-block 7-j; its Toeplitz window starts at q0+128j.
            ut2, u2s = {}, {}
            for hl in range(2):
                h = 2 * m + hl
                ut2[hl] = pu.tile([D, 512], F32, tag="u", name=f"u2_{h}{qch}")
                for j in range(NK):
                    nc.tensor.matmul(
                        ut2[hl],
                        lhsT=v16[:, j, h, :],
                        rhs=vbs[h][:, q0 + 128 * j:q0 + 128 * j + 512],
                        start=(j == 0), stop=(j == NK - 1),
                    )
            for hl in range(2):
                u2s[hl] = upool.tile([D, 512], BF16, tag="u2s",
                                     name=f"u2s{2 * m + hl}{qch}")
                nc.vector.tensor_copy(out=u2s[hl], in_=ut2[hl])
            ut1, u1s = {}, {}
            for hl in range(2):
                h = 2 * m + hl
                ut1[hl] = pu.tile([D + 1, 512], F32, tag="u", name=f"u1_{h}{qch}")
                for t in range(4):
                    nc.tensor.matmul(
                        ut1[hl], lhsT=v8[:, 2 * t:2 * t + 2, h, 0:D + 1],
                        rhs=e_pair[hl][:, 2 * t:2 * t + 2, :],
                        start=(t == 0), stop=(t == 3), perf_mode=DR,
                    )
            for hl in range(2):
                u1s[hl] = upool.tile([D + 1, 512], BF16, tag="u1s",
                                     name=f"u1s{2 * m + hl}{qch}")
                nc.vector.tensor_copy(out=u1s[hl], in_=ut1[hl])

            # per head: transpose bias term (cols 0-63) and PV term + Z
            # (cols 64-128) into one [128, 4, 130] psum tile, copy to sbuf,
            # then Pool does out = t1 * (1/Z) + t2 (all-sbuf operands).
            for hl in range(2):
                h = 2 * m + hl
                tt = pu.tile([128, 4, 2 * D + 2], BF16, tag="u",
                             name=f"tt_{h}{qch}")
                for ql in range(4):
                    nc.tensor.transpose(
                        tt[:, ql, 0:D],
                        u2s[hl][:, 128 * ql:128 * (ql + 1)],
                        ident[0:D, 0:D])
                    nc.tensor.transpose(
                        tt[:, ql, D:2 * D + 1],
                        u1s[hl][:, 128 * ql:128 * (ql + 1)],
                        ident[0:D + 1, 0:D + 1])
                tts = upool.tile([128, 4, 2 * D + 2], BF16, tag="tts",
                                 name=f"tts_{h}{qch}")
                nc.vector.tensor_copy(out=tts, in_=tt)
                rz = small.tile([128, 4], F32, tag="rz", name=f"rz{h}{qch}")
                nc.vector.reciprocal(
                    rz, tts[:, :, 2 * D:2 * D + 1].rearrange("p a b -> p (a b)"))
                c0 = 128 * m + 64 * hl
                for ql in range(4):
                    qb = 4 * qch + ql
                    nc.vector.scalar_tensor_tensor(
                        out=out_sb[:, qb, c0:c0 + D],
                        in0=tts[:, ql, D:2 * D], scalar=rz[:, ql:ql + 1],
                        in1=tts[:, ql, 0:D],
                        op0=mybir.AluOpType.mult, op1=mybir.AluOpType.add,
                    )
                    if last and hl == 1:
                        nc.vector.bn_stats(out=stats_sb[:, ql, :],
                                           in_=out_sb[:, qb, :])

        def emit_stats_cc(qch, stats_sb=None):
            cc_in, cc_out = cc[qch]
            if stats_sb is None:
                stats_sb = small.tile([128, 4, 6], F32, tag="stats",
                                      name=f"st_{qch}")
                for ql in range(4):
                    nc.vector.bn_stats(out=stats_sb[:, ql, :],
                                       in_=out_sb[:, 4 * qch + ql, :])
            nc.sync.dma_start(out=cc_in,
                              in_=stats_sb.rearrange("p a b -> p (a b)"))
            nc.gpsimd.collective_compute(
                kind="AllGather",
                op=mybir.AluOpType.bypass,
                replica_groups=[[0, 1], [2, 3], [4, 5], [6, 7]],
                ins=[cc_in], outs=[cc_out],
            )
            allst = small.tile([128, 4, 2, 6], F32, tag="allst", name=f"al{qch}")
            for r in range(2):
                nc.sync.dma_start(
                    out=allst[:, :, r, :],
                    in_=cc_out[128 * r:128 * (r + 1), :].rearrange(
                        "p (a b) -> p a b", b=6),
                )
            return allst

        def emit_ln(qch, allst):
            mv = small.tile([128, 4, 2], F32, tag="mv", name=f"mv{qch}")
            for ql in range(4):
                nc.vector.bn_aggr(out=mv[:, ql, :], in_=allst[:, ql, :, :])
            rstd = small.tile([128, 4], F32, tag="rstd", name=f"rs{qch}")
            nc.scalar.activation(
                out=rstd, in_=mv[:, :, 1:2].rearrange("p a b -> p (a b)"),
                func=mybir.ActivationFunctionType.Sqrt,
                bias=eps_t, scale=1.0,
            )
            nc.vector.reciprocal(rstd, rstd)
            negms = small.tile([128, 4], F32, tag="negms", name=f"ng{qch}")
            nc.vector.tensor_scalar(
                out=negms,
                in0=mv[:, :, 0:1].rearrange("p a b -> p (a b)"),
                scalar1=-1.0, scalar2=None, op0=mybir.AluOpType.mult,
            )
            nc.vector.tensor_tensor(
                out=negms, in0=negms, in1=rstd, op=mybir.AluOpType.mult)
            for ql in range(4):
                qb = 4 * qch + ql
                row = out_sb[:, qb, :]
                if ql % 2 == 0:
                    nc.vector.tensor_scalar(
                        out=row, in0=row,
                        scalar1=mv[:, ql, 0:1], scalar2=rstd[:, ql:ql + 1],
                        op0=mybir.AluOpType.subtract, op1=mybir.AluOpType.mult,
                    )
                else:
                    # (x - mu) * r == x * r + (-mu * r): ACT form, runs in
                    # parallel with DVE doing the even blocks
                    nc.scalar.activation(
                        out=row, in_=row,
                        func=mybir.ActivationFunctionType.Identity,
                        bias=negms[:, ql:ql + 1], scale=rstd[:, ql:ql + 1],
                    )
                nc.sync.dma_start(out=y[128 * qb:128 * (qb + 1), :], in_=row)

        # ---- main schedule: PV lags scores so ACT's exp stream never
        # starves (lag 2 in chunk 0 to cover the V projection, lag 1 after).
        e_saved = {}
        emit_kq(0)
        emit_kq(1)
        e_saved[(0, 0)] = emit_scores_exp(0, 0)
        emit_kq(2)
        e_saved[(1, 0)] = emit_scores_exp(1, 0)
        emit_kq(3)
        e_saved[(2, 0)] = emit_scores_exp(2, 0)
        emit_v(range(0, 4))
        e_saved[(3, 0)] = emit_scores_exp(3, 0)
        emit_v(range(4, 8))
        emit_pv(0, 0, e_saved.pop((0, 0)))
        emit_pv(1, 0, e_saved.pop((1, 0)))
        e_saved[(0, 1)] = emit_scores_exp(0, 1)
        emit_pv(2, 0, e_saved.pop((2, 0)))
        e_saved[(1, 1)] = emit_scores_exp(1, 1)
        emit_pv(3, 0, e_saved.pop((3, 0)))
        allst0 = emit_stats_cc(0)
        e_saved[(2, 1)] = emit_scores_exp(2, 1)
        emit_pv(0, 1, e_saved.pop((0, 1)))
        e_saved[(3, 1)] = emit_scores_exp(3, 1)
        emit_pv(1, 1, e_saved.pop((1, 1)))
        emit_pv(2, 1, e_saved.pop((2, 1)))
        stats1 = small.tile([128, 4, 6], F32, tag="stats", name="st_1")
        emit_pv(3, 1, e_saved.pop((3, 1)), last=True, stats_sb=stats1)
        allst1 = emit_stats_cc(1, stats_sb=stats1)  # last collective ASAP
        emit_ln(0, allst0)            # overlaps collective 1
        emit_ln(1, allst1)


def _to_fp8(a):
    return np.clip(a, -240.0, 240.0).astype(ml_dtypes.float8_e4m3fn)


def kernel(x, Wq, Wk, Wv, bias_table, ln_gamma, ln_beta):
    x = np.ascontiguousarray(np.asarray(x, np.float32))
    WqT = np.asarray(Wq, np.float32).T          # [E, E] (in, out)
    WkT = np.asarray(Wk, np.float32).T
    WvT = np.asarray(Wv, np.float32).T
    tblT = np.asarray(bias_table, np.float32).T  # [H, 2S-1]
    g = np.asarray(ln_gamma, np.float32)
    bta = np.asarray(ln_beta, np.float32)

    if "nc" not in _cache:
        _cache["nc"] = _build_nc()
    nc = _cache["nc"]

    xT = np.ascontiguousarray(x.transpose(0, 2, 1))          # [B, E, S]
    x8 = _to_fp8(xT)

    # Toeplitz windows, bf16: vb[h, p, u] = tblT[h, 127 + u - p]
    p_i = np.arange(128)[:, None]
    u_i = np.arange(VBW)[None, :]
    idx = 127 - p_i + u_i                                    # in [0, 2046]
    vb_all = tblT[:, idx].astype(ml_dtypes.bfloat16)         # [H, 128, VBW]

    def pack_kq(WT, sl):
        # [E, 512] -> [4, 128, NK, 128]: w[m, p, k, j] = WT[128k+p, sl0+128m+j]
        # (partitions = contraction e, free = output columns)
        A = _to_fp8(WS * WT[:, sl])
        return np.ascontiguousarray(
            A.reshape(NK, 128, 4, 128).transpose(2, 1, 0, 3))

    def pack_wv(WT, sl):
        # [E, 512] -> [128, NK, 512]: w[p, k, j] = WT[128k+p, sl0+j]
        A = np.asarray(WT[:, sl], np.float32)
        return np.ascontiguousarray(A.reshape(NK, 128, EC).transpose(1, 0, 2))

    in_maps = []
    for c in range(8):
        b, hg = c // 2, c % 2
        sl = slice(EC * hg, EC * (hg + 1))
        wk8 = pack_kq(WkT, sl)
        wq8 = pack_kq(WqT, sl)
        in_maps.append({
            "xT": xT[b],
            "x8": x8[b],
            "wkq8": np.ascontiguousarray(np.concatenate([wk8, wq8], axis=0)),
            "wv": pack_wv(WvT, sl),
            "tbl16": np.ascontiguousarray(vb_all[HC * hg: HC * hg + HC]),
        })

    res = run_bass_kernel_spmd(nc, in_maps, core_ids=list(range(8)))
    _cache["last_results"] = res

    out = np.empty((B, S, E), np.float32)
    for c in range(8):
        b, hg = c // 2, c % 2
        out[b, :, EC * hg: EC * (hg + 1)] = res.results[c]["y"]
    # gamma/beta are ones/zeros in this problem; apply on host if not.
    if not (np.all(g == 1.0) and np.all(bta == 0.0)):
        out = out * g + bta
    return out
